# revision 22
# baseline (speedup 1.0000x reference)
"""GQA causal attention (B=2,S=2048,D=4096,NH=32,NKV=8,HD=128) on 8 TRN2 cores.

Sharding: core c -> batch b=c//4, kv-group g=c%4 (2 kv heads, 8 q heads).
Each core computes a partial output x_b @ (its heads) @ wo_rows; host sums
the 4 partials per batch.

Device dataflow (bf16 matmuls, fp32 PSUM/softmax stats):
  phase 1: xT chunks [128,512] stream in; Q/K projections produce qT/kT in
           [HD,S] layout (+RoPE via a swap-permutation matmul and two DVE
           multiplies against [cos;cos], [-sin;sin] tables); V in [S,HD].
  phase 2: scores computed TRANSPOSED per 512-wide sq block:
           S_T[t,sq] = kT_tile.T @ qT_block, causal mask added from 4
           precomputed [128,512] tiles, exp -> P_T (bf16), rowsums via
           ones-vector matmul, attnT[d,sq] += (lhsT=V_tile) @ P_T,
           normalization = rank-1 (ones x 1/rowsum) matmul broadcast +
           one DVE multiply on the attnT PSUM->SBUF copy.
  phase 3: out[sq,:] = sum_h attnT_h.T @ wo_h, streamed to DRAM.
"""

import os
import sys

import numpy as np
import ml_dtypes

for _p in ("/opt/trn_rl_repo",):
    if os.path.isdir(_p) and _p not in sys.path:
        sys.path.insert(0, _p)

B, S, D = 2, 2048, 4096
NH, NKV, HD = 32, 8, 128
NCORE = 8
HQ = 8           # q heads per core
HKV = 2          # kv heads per core
NDC = D // 128   # 32 contraction chunks
SB = 512         # phase-1 seq block
NBLK = S // SB
NT = S // 128    # 16 sq tiles of 128
NB2 = S // 512   # phase-2 sq blocks of 512
SCALE = 1.0 / float(np.sqrt(HD))

_BF16 = ml_dtypes.bfloat16
_GRAPH = None


def _maybe_patch_ldw_opt():
    if os.environ.get("ATTN_LDW_OPT") != "1":
        return
    from concourse import bass_utils as _bu
    if getattr(_bu, "_ldw_patched", False):
        return
    _orig = _bu.run_command

    def _patched(argv, **kw):
        argv = ["--enable-ldw-opt=true" if a == "--enable-ldw-opt=false" else a
                for a in argv]
        return _orig(argv, **kw)

    _bu.run_command = _patched
    _bu._ldw_patched = True


def _build_graph():
    _maybe_patch_ldw_opt()
    import concourse.mybir as mybir
    import concourse.tile as tile
    from concourse import bacc

    f32 = mybir.dt.float32
    bf16 = mybir.dt.bfloat16
    AF = mybir.ActivationFunctionType
    OP = mybir.AluOpType

    nc = bacc.Bacc("TRN2", target_bir_lowering=False, debug=False)
    xT_d = nc.declare_dram_parameter("xT", [D, S], bf16, isOutput=False)
    wq_d = nc.declare_dram_parameter("wq", [128, NDC * HQ * HD], bf16, isOutput=False)
    wk_d = nc.declare_dram_parameter("wk", [128, NDC * HKV * HD], bf16, isOutput=False)
    wv_d = nc.declare_dram_parameter("wv", [128, NDC * HKV * HD], bf16, isOutput=False)
    wo_d = nc.declare_dram_parameter("wo", [128, HQ * D], bf16, isOutput=False)
    cos_d = nc.declare_dram_parameter("cosT", [128, S], bf16, isOutput=False)
    sin_d = nc.declare_dram_parameter("sinT", [128, S], bf16, isOutput=False)
    cm_d = nc.declare_dram_parameter("cmask", [128, 4 * 512], f32, isOutput=False)
    o16_d = nc.declare_dram_parameter("ones16", [128, 128], bf16, isOutput=False)
    o32_d = nc.declare_dram_parameter("ones32", [1, 128], f32, isOutput=False)
    swp_d = nc.declare_dram_parameter("swp", [128, 128], bf16, isOutput=False)
    out_d = nc.declare_dram_parameter("out", [D, S], f32, isOutput=True)

    with tile.TileContext(nc) as tc:
        with tc.tile_pool(name="res", bufs=1) as res:
            swp = res.tile([128, 128], bf16, tag="swp")
            nc.gpsimd.dma_start(out=swp[:], in_=swp_d[:])
            ones16 = res.tile([128, 128], bf16, tag="ones16")
            nc.gpsimd.dma_start(out=ones16[:], in_=o16_d[:])
            ones32 = res.tile([1, 128], f32, tag="ones32")
            nc.gpsimd.dma_start(out=ones32[:], in_=o32_d[:])
            res_qkv = tc.alloc_tile_pool(name="res_qkv", bufs=1)
            qT = [res_qkv.tile([128, S], bf16, tag=f"qT{h}", name=f"qT{h}") for h in range(HQ)]
            kT = [res_qkv.tile([128, S], bf16, tag=f"kT{g}", name=f"kT{g}") for g in range(HKV)]
            V = [res_qkv.tile([128, S], bf16, tag=f"V{g}", name=f"V{g}") for g in range(HKV)]

            # ---------------- Phase 1: QKV projections + RoPE ----------------
            with tc.tile_pool(name="p1w", bufs=1) as p1w, \
                 tc.tile_pool(name="p1x", bufs=33) as p1x, \
                 tc.tile_pool(name="p1s", bufs=2) as p1s, \
                 tc.tile_pool(name="ps1", bufs=3, space="PSUM") as ps1:
                for blk in range(NBLK):
                    s0 = blk * SB
                    xts = []
                    for dc in range(NDC):
                        t = p1x.tile([128, SB], bf16, tag="xT", name=f"xt{blk}_{dc}")
                        nc.sync.dma_start(
                            out=t[:], in_=xT_d[dc * 128:(dc + 1) * 128, s0:s0 + SB])
                        xts.append(t)
                    if blk == 0:
                        wq_sb = p1w.tile([128, NDC * HQ * HD], bf16, tag="wq")
                        qn = NDC * HQ * HD // 4
                        for q in range(4):
                            nc.gpsimd.dma_start(out=wq_sb[:, q * qn:(q + 1) * qn],
                                                in_=wq_d[:, q * qn:(q + 1) * qn])
                        wk_sb = p1w.tile([128, NDC * HKV * HD], bf16, tag="wk")
                        nc.scalar.dma_start(out=wk_sb[:], in_=wk_d[:])
                        wv_sb = p1w.tile([128, NDC * HKV * HD], bf16, tag="wv")
                        nc.scalar.dma_start(out=wv_sb[:], in_=wv_d[:])
                        cosb = p1w.tile([128, S], bf16, tag="cos")
                        nc.scalar.dma_start(out=cosb[:], in_=cos_d[:])
                        sinb = p1w.tile([128, S], bf16, tag="sin")
                        nc.scalar.dma_start(out=sinb[:], in_=sin_d[:])

                    # Q and K projections (transposed output [HD, seq]) + rope
                    for h in range(HQ + HKV):
                        ps = ps1.tile([128, SB], f32, tag="pp", bufs=3)
                        for dc in range(NDC):
                            if h < HQ:
                                w_sl = wq_sb[:, (dc * HQ + h) * 128:(dc * HQ + h + 1) * 128]
                            else:
                                gg = h - HQ
                                w_sl = wk_sb[:, (dc * HKV + gg) * 128:(dc * HKV + gg + 1) * 128]
                            nc.tensor.matmul(ps[:], w_sl, xts[dc][:],
                                             start=(dc == 0), stop=(dc == NDC - 1))
                        tmp = p1s.tile([128, SB], bf16, tag="ropetmp")
                        nc.vector.tensor_copy(tmp[:], ps[:])
                        qsw_ps = ps1.tile([128, SB], f32, tag="qsw", bufs=2)
                        nc.tensor.matmul(qsw_ps[:], swp[:], tmp[:], start=True, stop=True)
                        dst = qT[h] if h < HQ else kT[h - HQ]
                        dsl = dst[:, s0:s0 + SB]
                        ra = p1s.tile([128, SB], bf16, tag="ra")
                        rb = p1s.tile([128, SB], bf16, tag="rb")
                        nc.vector.tensor_tensor(ra[:], tmp[:], cosb[:, s0:s0 + SB], op=OP.mult)
                        nc.vector.tensor_tensor(rb[:], qsw_ps[:], sinb[:, s0:s0 + SB], op=OP.mult)
                        nc.vector.tensor_tensor(dsl[:], ra[:], rb[:], op=OP.add)
                    # V projection (natural [seq, d] layout)
                    for st in range(SB // 128):
                        ps = ps1.tile([128, HKV * HD], f32, tag="pv", bufs=2)
                        for dc in range(NDC):
                            nc.tensor.matmul(ps[:], xts[dc][:, st * 128:(st + 1) * 128],
                                             wv_sb[:, dc * 256:(dc + 1) * 256],
                                             start=(dc == 0), stop=(dc == NDC - 1))
                        sg = blk * (SB // 128) + st
                        for g in range(HKV):
                            nc.vector.tensor_copy(
                                V[g][:, sg * 128:(sg + 1) * 128],
                                ps[:, g * 128:(g + 1) * 128])

            # -------- Phase 2+3: attention (transposed scores) + out proj ---
            res_a = tc.alloc_tile_pool(name="res_a", bufs=1, side="right")
            attnT = [res_a.tile([128, S], bf16, tag=f"aT{h}", name=f"aT{h}")
                     for h in range(HQ)]
            with tc.tile_pool(name="p2c", bufs=1) as p2c, \
                 tc.tile_pool(name="p2", bufs=68) as p2, \
                 tc.tile_pool(name="p2s", bufs=4) as p2s, \
                 tc.tile_pool(name="p3w", bufs=4) as p3w, \
                 tc.tile_pool(name="p3o", bufs=4) as p3o, \
                 tc.tile_pool(name="pss", bufs=3, space="PSUM") as pss, \
                 tc.tile_pool(name="psr", bufs=1, space="PSUM") as psr, \
                 tc.tile_pool(name="psa", bufs=2, space="PSUM") as psa, \
                 tc.tile_pool(name="pso", bufs=2, space="PSUM") as pso:
                cmask = p2c.tile([128, 4 * 512], f32, tag="cmask")
                nc.gpsimd.dma_start(out=cmask[:], in_=cm_d[:])
                for b in range(NB2):
                    ntt = 4 * (b + 1)
                    sq0 = b * 512
                    for g in range(HKV):
                        pts_h = {}
                        for hh in range(4):
                            h = g * 4 + hh
                            pts = []
                            for tt in range(ntt):
                                st_ps = pss.tile([128, 512], f32, tag="sc")
                                nc.tensor.matmul(st_ps[:],
                                                 kT[g][:, tt * 128:(tt + 1) * 128],
                                                 qT[h][:, sq0:sq0 + 512],
                                                 start=True, stop=True)
                                j = tt - 4 * b
                                if j >= 0:
                                    nc.vector.tensor_tensor(
                                        st_ps[:], st_ps[:],
                                        cmask[:, j * 512:(j + 1) * 512], op=OP.add)
                                pt = p2.tile([128, 512], bf16, tag="pt",
                                             name=f"pt{hh}_{tt}")
                                nc.scalar.activation(pt[:], st_ps[:], AF.Exp,
                                                     bias=0.0, scale=SCALE)
                                pts.append(pt)
                            pts_h[hh] = pts
                        for hh in range(4):
                            h = g * 4 + hh
                            pts = pts_h[hh]
                            rs_ps = psr.tile([1, 512], f32, tag="rs")
                            for tt in range(ntt):
                                nc.tensor.matmul(rs_ps[:], ones16[:, 0:1], pts[tt][:],
                                                 start=(tt == 0), stop=(tt == ntt - 1))
                            at_ps = psa.tile([128, 512], f32, tag="at")
                            for tt in range(ntt):
                                nc.tensor.matmul(at_ps[:],
                                                 V[g][:, tt * 128:(tt + 1) * 128],
                                                 pts[tt][:],
                                                 start=(tt == 0), stop=(tt == ntt - 1))
                            # normalize: broadcast rowsum, reciprocal, multiply
                            rs_sb = p2s.tile([1, 512], f32, tag="rss")
                            nc.vector.tensor_copy(rs_sb[:], rs_ps[:])
                            bc_ps = pso.tile([128, 512], f32, tag="po", name="bc")
                            nc.tensor.matmul(bc_ps[:], ones32[:], rs_sb[:],
                                             start=True, stop=True)
                            rec = p2s.tile([128, 512], f32, tag="rec")
                            nc.vector.reciprocal(rec[:], bc_ps[:])
                            nc.vector.tensor_tensor(
                                attnT[h][:, sq0:sq0 + 512], at_ps[:], rec[:],
                                op=OP.mult)
                    # out projection for this sq block (all heads now final)
                    for n1 in range(D // 128):
                        wt = p3w.tile([128, HQ * HD], bf16, tag="wot",
                                      name=f"wot{n1}", bufs=8)
                        nc.gpsimd.dma_start(
                            out=wt[:], in_=wo_d[:, n1 * 1024:(n1 + 1) * 1024])
                        ps = pso.tile([128, 512], f32, tag="po")
                        for h in range(HQ):
                            nc.tensor.matmul(ps[:],
                                             wt[:, h * 128:(h + 1) * 128],
                                             attnT[h][:, sq0:sq0 + 512],
                                             start=(h == 0), stop=(h == HQ - 1))
                        ob = p3o.tile([128, 512], f32, tag="ob")
                        nc.vector.tensor_copy(ob[:], ps[:])
                        nc.sync.dma_start(
                            out=out_d[n1 * 128:(n1 + 1) * 128, sq0:sq0 + 512],
                            in_=ob[:])
            res_qkv.release()
            res_a.release()
    nc.compile()
    return nc


def _graph():
    global _GRAPH
    if _GRAPH is None:
        _GRAPH = _build_graph()
    return _GRAPH


_PERM = np.concatenate([np.arange(0, HD, 2), np.arange(1, HD, 2)])


def _make_in_maps(x, freqs_cos, freqs_sin, wq, wk, wv, wo):
    # deinterleave head-dim so rope pairs are (j, j+64) on partitions
    wq_p = wq.reshape(D, NH, HD)[:, :, _PERM]
    wk_p = wk.reshape(D, NKV, HD)[:, :, _PERM]
    cosT = np.concatenate([freqs_cos.T, freqs_cos.T], axis=0)
    cosT = np.ascontiguousarray(cosT).astype(_BF16)
    sinT = np.concatenate([-freqs_sin.T, freqs_sin.T], axis=0)
    sinT = np.ascontiguousarray(sinT).astype(_BF16)
    r = np.arange(128)[:, None]
    c = np.arange(512)[None, :]
    cmask = np.concatenate(
        [np.where(c < j * 128 + r, np.float32(-1e30), np.float32(0.0))
         for j in range(4)], axis=1).astype(np.float32)
    ones16 = np.ones((128, 128), dtype=_BF16)
    ones32 = np.ones((1, 128), dtype=np.float32)
    swp = np.zeros((128, 128), np.float32)
    swp[np.arange(128), (np.arange(128) + 64) % 128] = 1.0
    swp = swp.astype(_BF16)
    xTs = [np.ascontiguousarray(x[b].T).astype(_BF16) for b in range(B)]
    in_maps = []
    for core in range(NCORE):
        bb, g4 = core // 4, core % 4
        hq0, kv0 = g4 * 8, g4 * 2
        # weight retiling: [128, dc-major, head, col] contiguous per partition
        wq_t = wq_p[:, hq0:hq0 + HQ].reshape(NDC, 128, HQ * HD)
        wq_t = np.ascontiguousarray(wq_t.transpose(1, 0, 2)).reshape(128, NDC * HQ * HD)
        wk_t = wk_p[:, kv0:kv0 + HKV].reshape(NDC, 128, HKV * HD)
        wk_t = np.ascontiguousarray(wk_t.transpose(1, 0, 2)).reshape(128, NDC * HKV * HD)
        wv_t = wv[:, kv0 * HD:(kv0 + HKV) * HD].reshape(NDC, 128, HKV * HD)
        wv_t = np.ascontiguousarray(wv_t.transpose(1, 0, 2)).reshape(128, NDC * HKV * HD)
        # [p, n1, h, c]: per n1-tile, all 8 heads' [128,128] blocks contiguous
        wo_t = wo[hq0 * HD:(hq0 + HQ) * HD, :].reshape(HQ, 128, D // 128, 128)
        wo_t = np.ascontiguousarray(wo_t.transpose(1, 2, 0, 3)).reshape(128, D // 128 * HQ * 128)
        in_maps.append({
            "xT": xTs[bb],
            "wq": wq_t.astype(_BF16),
            "wk": wk_t.astype(_BF16),
            "wv": wv_t.astype(_BF16),
            "wo": wo_t.astype(_BF16),
            "cosT": cosT,
            "sinT": sinT,
            "cmask": cmask,
            "ones16": ones16,
            "ones32": ones32,
            "swp": swp,
        })
    return in_maps


def _run(in_maps, trace=False, tmpdir=None):
    from concourse.bass_utils import run_bass_kernel_spmd
    return run_bass_kernel_spmd(_graph(), in_maps, core_ids=list(range(NCORE)),
                                trace=trace, tmpdir=tmpdir)


def kernel(x, freqs_cos, freqs_sin, wq, wk, wv, wo):
    in_maps = _make_in_maps(np.asarray(x, np.float32),
                            np.asarray(freqs_cos, np.float32),
                            np.asarray(freqs_sin, np.float32),
                            np.asarray(wq, np.float32),
                            np.asarray(wk, np.float32),
                            np.asarray(wv, np.float32),
                            np.asarray(wo, np.float32))
    res = _run(in_maps).results
    outT = np.zeros((B, D, S), np.float32)
    for c in range(NCORE):
        outT[c // 4] += np.asarray(res[c]["out"], np.float32)
    return np.ascontiguousarray(outT.transpose(0, 2, 1))


def _ensure_ntff_hook():
    import types
    import ctypes
    import contextlib
    import antenv

    try:
        from antenv.axon_hooks import get_axon_ntff_profile_hook  # noqa: F401
        return
    except ImportError:
        pass
    so_path = "/opt/axon/libaxon_pjrt.so"
    if not os.path.exists(so_path):
        return
    lib = ctypes.CDLL(so_path)
    if not hasattr(lib, "axon_start_nrt_profile"):
        return
    lib.axon_start_nrt_profile.argtypes = [ctypes.POINTER(ctypes.c_int64),
                                           ctypes.c_size_t]
    lib.axon_start_nrt_profile.restype = ctypes.c_int64
    lib.axon_stop_nrt_profile.argtypes = [ctypes.c_char_p]
    lib.axon_stop_nrt_profile.restype = ctypes.c_int64

    @contextlib.contextmanager
    def _hook(output_dir, device_ids):
        import jax
        jax.devices()
        if device_ids:
            ids = (ctypes.c_int64 * len(device_ids))(*device_ids)
            rc = lib.axon_start_nrt_profile(ids, len(device_ids))
        else:
            rc = lib.axon_start_nrt_profile(None, 0)
        if rc != 0:
            raise RuntimeError(f"axon_start_nrt_profile rc={rc}")
        try:
            yield
        finally:
            n = lib.axon_stop_nrt_profile(str(output_dir).encode())
            print(f"profile: {n} ntff file(s) written to {output_dir}")

    mod = types.ModuleType("antenv.axon_hooks")
    hooks = {"h": _hook}
    mod.get_axon_ntff_profile_hook = lambda: hooks["h"]
    mod.set_axon_ntff_profile_hook = lambda h: hooks.__setitem__("h", h)
    sys.modules["antenv.axon_hooks"] = mod
    antenv.axon_hooks = mod


def profile_exec_ns(inputs, tmpdir=None):
    _ensure_ntff_hook()
    from concourse import bass_utils as _bu
    _bu.upload_artifacts = lambda d: f"local:{d}"
    in_maps = _make_in_maps(**{k: np.asarray(v, np.float32) for k, v in inputs.items()})
    r = _run(in_maps, trace=True, tmpdir=tmpdir)
    return r.exec_time_ns


# revision 23
# speedup vs baseline: 1.1636x; 1.1636x over previous
"""GQA causal attention (B=2,S=2048,D=4096,NH=32,NKV=8,HD=128) on 8 TRN2 cores.

Sharding: core c -> batch b=c//4, kv-group g=c%4 (2 kv heads, 8 q heads).
Each core computes a partial output x_b @ (its heads) @ wo_rows; host sums
the 4 partials per batch.

Device dataflow (bf16 matmuls, fp32 PSUM/softmax stats):
  phase 1: xT chunks [128,512] stream in; Q/K projections produce qT/kT in
           [HD,S] layout (+RoPE via a swap-permutation matmul and two DVE
           multiplies against [cos;cos], [-sin;sin] tables); V in [S,HD].
  phase 2: scores computed TRANSPOSED per 512-wide sq block:
           S_T[t,sq] = kT_tile.T @ qT_block, causal mask added from 4
           precomputed [128,512] tiles, exp -> P_T (bf16), rowsums via
           ones-vector matmul, attnT[d,sq] += (lhsT=V_tile) @ P_T,
           normalization = rank-1 (ones x 1/rowsum) matmul broadcast +
           one DVE multiply on the attnT PSUM->SBUF copy.
  phase 3: out[sq,:] = sum_h attnT_h.T @ wo_h, streamed to DRAM.
"""

import os
import sys

import numpy as np
import ml_dtypes

for _p in ("/opt/trn_rl_repo",):
    if os.path.isdir(_p) and _p not in sys.path:
        sys.path.insert(0, _p)

B, S, D = 2, 2048, 4096
NH, NKV, HD = 32, 8, 128
NCORE = 8
HQ = 8           # q heads per core
HKV = 2          # kv heads per core
NDC = D // 128   # 32 contraction chunks
SB = 512         # phase-1 seq block
NBLK = S // SB
NT = S // 128    # 16 sq tiles of 128
NB2 = S // 512   # phase-2 sq blocks of 512
SCALE = 1.0 / float(np.sqrt(HD))

_BF16 = ml_dtypes.bfloat16
_GRAPH = None


def _maybe_patch_ldw_opt():
    if os.environ.get("ATTN_LDW_OPT") != "1":
        return
    from concourse import bass_utils as _bu
    if getattr(_bu, "_ldw_patched", False):
        return
    _orig = _bu.run_command

    def _patched(argv, **kw):
        argv = ["--enable-ldw-opt=true" if a == "--enable-ldw-opt=false" else a
                for a in argv]
        return _orig(argv, **kw)

    _bu.run_command = _patched
    _bu._ldw_patched = True


def _build_graph():
    _maybe_patch_ldw_opt()
    import concourse.mybir as mybir
    import concourse.tile as tile
    from concourse import bacc

    f32 = mybir.dt.float32
    bf16 = mybir.dt.bfloat16
    AF = mybir.ActivationFunctionType
    OP = mybir.AluOpType

    nc = bacc.Bacc("TRN2", target_bir_lowering=False, debug=False)
    xT_d = nc.declare_dram_parameter("xT", [D, S], bf16, isOutput=False)
    wq_d = nc.declare_dram_parameter("wq", [128, NDC * HQ * HD], bf16, isOutput=False)
    wk_d = nc.declare_dram_parameter("wk", [128, NDC * HKV * HD], bf16, isOutput=False)
    wv_d = nc.declare_dram_parameter("wv", [128, NDC * HKV * HD], bf16, isOutput=False)
    wo_d = nc.declare_dram_parameter("wo", [128, HQ * D], bf16, isOutput=False)
    cos_d = nc.declare_dram_parameter("cosT", [128, S], bf16, isOutput=False)
    sin_d = nc.declare_dram_parameter("sinT", [128, S], bf16, isOutput=False)
    cm_d = nc.declare_dram_parameter("cmask", [128, 4 * 512], f32, isOutput=False)
    o16_d = nc.declare_dram_parameter("ones16", [128, 128], bf16, isOutput=False)
    o32_d = nc.declare_dram_parameter("ones32", [1, 128], f32, isOutput=False)
    swp_d = nc.declare_dram_parameter("swp", [128, 128], bf16, isOutput=False)
    out_d = nc.declare_dram_parameter("out", [D, S], f32, isOutput=True)

    with tile.TileContext(nc) as tc:
        with tc.tile_pool(name="res", bufs=1) as res:
            swp = res.tile([128, 128], bf16, tag="swp")
            nc.gpsimd.dma_start(out=swp[:], in_=swp_d[:])
            ones16 = res.tile([128, 128], bf16, tag="ones16")
            nc.gpsimd.dma_start(out=ones16[:], in_=o16_d[:])
            ones32 = res.tile([1, 128], f32, tag="ones32")
            nc.gpsimd.dma_start(out=ones32[:], in_=o32_d[:])
            res_qkv = tc.alloc_tile_pool(name="res_qkv", bufs=1)
            qT = [res_qkv.tile([128, S], bf16, tag=f"qT{h}", name=f"qT{h}") for h in range(HQ)]
            kT = [res_qkv.tile([128, S], bf16, tag=f"kT{g}", name=f"kT{g}") for g in range(HKV)]
            V = [res_qkv.tile([128, S], bf16, tag=f"V{g}", name=f"V{g}") for g in range(HKV)]

            # ---------------- Phase 1: QKV projections + RoPE ----------------
            with tc.tile_pool(name="p1w", bufs=1) as p1w, \
                 tc.tile_pool(name="p1x", bufs=33) as p1x, \
                 tc.tile_pool(name="p1s", bufs=2) as p1s, \
                 tc.tile_pool(name="ps1", bufs=3, space="PSUM") as ps1:
                for blk in range(NBLK):
                    s0 = blk * SB
                    xts = []
                    for dc in range(NDC):
                        t = p1x.tile([128, SB], bf16, tag="xT", name=f"xt{blk}_{dc}")
                        nc.sync.dma_start(
                            out=t[:], in_=xT_d[dc * 128:(dc + 1) * 128, s0:s0 + SB])
                        xts.append(t)
                    if blk == 0:
                        wq_sb = p1w.tile([128, NDC * HQ * HD], bf16, tag="wq")
                        qn = NDC * HQ * HD // 4
                        for q in range(4):
                            nc.gpsimd.dma_start(out=wq_sb[:, q * qn:(q + 1) * qn],
                                                in_=wq_d[:, q * qn:(q + 1) * qn])
                        wk_sb = p1w.tile([128, NDC * HKV * HD], bf16, tag="wk")
                        nc.scalar.dma_start(out=wk_sb[:], in_=wk_d[:])
                        wv_sb = p1w.tile([128, NDC * HKV * HD], bf16, tag="wv")
                        nc.scalar.dma_start(out=wv_sb[:], in_=wv_d[:])
                        cosb = p1w.tile([128, S], bf16, tag="cos")
                        nc.scalar.dma_start(out=cosb[:], in_=cos_d[:])
                        sinb = p1w.tile([128, S], bf16, tag="sin")
                        nc.scalar.dma_start(out=sinb[:], in_=sin_d[:])

                    # Q and K projections (transposed output [HD, seq]) + rope
                    for h in range(HQ + HKV):
                        ps = ps1.tile([128, SB], f32, tag="pp", bufs=3)
                        for dc in range(NDC):
                            if h < HQ:
                                w_sl = wq_sb[:, (dc * HQ + h) * 128:(dc * HQ + h + 1) * 128]
                            else:
                                gg = h - HQ
                                w_sl = wk_sb[:, (dc * HKV + gg) * 128:(dc * HKV + gg + 1) * 128]
                            nc.tensor.matmul(ps[:], w_sl, xts[dc][:],
                                             start=(dc == 0), stop=(dc == NDC - 1))
                        tmp = p1s.tile([128, SB], bf16, tag="ropetmp")
                        nc.vector.tensor_copy(tmp[:], ps[:])
                        qsw_ps = ps1.tile([128, SB], f32, tag="qsw", bufs=2)
                        nc.tensor.matmul(qsw_ps[:], swp[:], tmp[:], start=True, stop=True)
                        dst = qT[h] if h < HQ else kT[h - HQ]
                        dsl = dst[:, s0:s0 + SB]
                        ra = p1s.tile([128, SB], bf16, tag="ra")
                        rb = p1s.tile([128, SB], bf16, tag="rb")
                        nc.vector.tensor_tensor(ra[:], tmp[:], cosb[:, s0:s0 + SB], op=OP.mult)
                        nc.vector.tensor_tensor(rb[:], qsw_ps[:], sinb[:, s0:s0 + SB], op=OP.mult)
                        nc.vector.tensor_tensor(dsl[:], ra[:], rb[:], op=OP.add)
                    # V projection (natural [seq, d] layout)
                    for st in range(SB // 128):
                        ps = ps1.tile([128, HKV * HD], f32, tag="pv", bufs=2)
                        for dc in range(NDC):
                            nc.tensor.matmul(ps[:], xts[dc][:, st * 128:(st + 1) * 128],
                                             wv_sb[:, dc * 256:(dc + 1) * 256],
                                             start=(dc == 0), stop=(dc == NDC - 1))
                        sg = blk * (SB // 128) + st
                        for g in range(HKV):
                            nc.vector.tensor_copy(
                                V[g][:, sg * 128:(sg + 1) * 128],
                                ps[:, g * 128:(g + 1) * 128])

            # -------- Phase 2+3: attention (transposed scores) + out proj ---
            res_a = tc.alloc_tile_pool(name="res_a", bufs=1, side="right")
            attnT = [res_a.tile([128, S], bf16, tag=f"aT{h}", name=f"aT{h}")
                     for h in range(HQ)]
            with tc.tile_pool(name="p2c", bufs=1) as p2c, \
                 tc.tile_pool(name="p2", bufs=68) as p2, \
                 tc.tile_pool(name="p2s", bufs=4) as p2s, \
                 tc.tile_pool(name="p3w", bufs=4) as p3w, \
                 tc.tile_pool(name="p3o", bufs=4) as p3o, \
                 tc.tile_pool(name="pss", bufs=3, space="PSUM") as pss, \
                 tc.tile_pool(name="psa", bufs=2, space="PSUM") as psa, \
                 tc.tile_pool(name="psb", bufs=1, space="PSUM") as psb, \
                 tc.tile_pool(name="pso", bufs=2, space="PSUM") as pso:
                cmask = p2c.tile([128, 4 * 512], f32, tag="cmask")
                nc.gpsimd.dma_start(out=cmask[:], in_=cm_d[:])
                for b in range(NB2):
                    ntt = 4 * (b + 1)
                    sq0 = b * 512
                    for g in range(HKV):
                        pts_h = {}
                        for hh in range(4):
                            h = g * 4 + hh
                            pts = []
                            for tt in range(ntt):
                                st_ps = pss.tile([128, 512], f32, tag="sc")
                                nc.tensor.matmul(st_ps[:],
                                                 kT[g][:, tt * 128:(tt + 1) * 128],
                                                 qT[h][:, sq0:sq0 + 512],
                                                 start=True, stop=True)
                                j = tt - 4 * b
                                if j >= 0:
                                    nc.vector.tensor_tensor(
                                        st_ps[:], st_ps[:],
                                        cmask[:, j * 512:(j + 1) * 512], op=OP.add)
                                pt = p2.tile([128, 512], bf16, tag="pt",
                                             name=f"pt{hh}_{tt}")
                                nc.scalar.activation(pt[:], st_ps[:], AF.Exp,
                                                     bias=0.0, scale=SCALE)
                                pts.append(pt)
                            pts_h[hh] = pts
                        for hh in range(4):
                            h = g * 4 + hh
                            pts = pts_h[hh]
                            # partial row-sums on DVE, one tiny matmul reduces
                            acc = p2s.tile([128, 512], bf16, tag="acc")
                            nc.vector.tensor_copy(acc[:], pts[0][:])
                            for tt in range(1, ntt):
                                nc.vector.tensor_tensor(acc[:], acc[:], pts[tt][:],
                                                        op=OP.add)
                            at_ps = psa.tile([128, 512], f32, tag="at")
                            for tt in range(ntt):
                                nc.tensor.matmul(at_ps[:],
                                                 V[g][:, tt * 128:(tt + 1) * 128],
                                                 pts[tt][:],
                                                 start=(tt == 0), stop=(tt == ntt - 1))
                            # normalize: broadcast rowsum, reciprocal, multiply
                            bc_ps = psb.tile([128, 512], f32, tag="bc")
                            nc.tensor.matmul(bc_ps[0:1, :], ones16[:, 0:1], acc[:],
                                             start=True, stop=True)
                            rs_sb = p2s.tile([1, 512], f32, tag="rss")
                            nc.vector.tensor_copy(rs_sb[:], bc_ps[0:1, :])
                            nc.tensor.matmul(bc_ps[:], ones32[:], rs_sb[:],
                                             start=True, stop=True)
                            rec = p2s.tile([128, 512], f32, tag="rec")
                            nc.vector.reciprocal(rec[:], bc_ps[:])
                            nc.vector.tensor_tensor(
                                attnT[h][:, sq0:sq0 + 512], at_ps[:], rec[:],
                                op=OP.mult)
                    # out projection for this sq block (all heads now final)
                    for n1 in range(D // 128):
                        wt = p3w.tile([128, HQ * HD], bf16, tag="wot",
                                      name=f"wot{n1}", bufs=8)
                        nc.gpsimd.dma_start(
                            out=wt[:], in_=wo_d[:, n1 * 1024:(n1 + 1) * 1024])
                        ps = pso.tile([128, 512], f32, tag="po")
                        for h in range(HQ):
                            nc.tensor.matmul(ps[:],
                                             wt[:, h * 128:(h + 1) * 128],
                                             attnT[h][:, sq0:sq0 + 512],
                                             start=(h == 0), stop=(h == HQ - 1))
                        ob = p3o.tile([128, 512], f32, tag="ob")
                        nc.vector.tensor_copy(ob[:], ps[:])
                        nc.sync.dma_start(
                            out=out_d[n1 * 128:(n1 + 1) * 128, sq0:sq0 + 512],
                            in_=ob[:])
            res_qkv.release()
            res_a.release()
    nc.compile()
    return nc


def _graph():
    global _GRAPH
    if _GRAPH is None:
        _GRAPH = _build_graph()
    return _GRAPH


_PERM = np.concatenate([np.arange(0, HD, 2), np.arange(1, HD, 2)])


def _make_in_maps(x, freqs_cos, freqs_sin, wq, wk, wv, wo):
    # deinterleave head-dim so rope pairs are (j, j+64) on partitions
    wq_p = wq.reshape(D, NH, HD)[:, :, _PERM]
    wk_p = wk.reshape(D, NKV, HD)[:, :, _PERM]
    cosT = np.concatenate([freqs_cos.T, freqs_cos.T], axis=0)
    cosT = np.ascontiguousarray(cosT).astype(_BF16)
    sinT = np.concatenate([-freqs_sin.T, freqs_sin.T], axis=0)
    sinT = np.ascontiguousarray(sinT).astype(_BF16)
    r = np.arange(128)[:, None]
    c = np.arange(512)[None, :]
    cmask = np.concatenate(
        [np.where(c < j * 128 + r, np.float32(-1e30), np.float32(0.0))
         for j in range(4)], axis=1).astype(np.float32)
    ones16 = np.ones((128, 128), dtype=_BF16)
    ones32 = np.ones((1, 128), dtype=np.float32)
    swp = np.zeros((128, 128), np.float32)
    swp[np.arange(128), (np.arange(128) + 64) % 128] = 1.0
    swp = swp.astype(_BF16)
    xTs = [np.ascontiguousarray(x[b].T).astype(_BF16) for b in range(B)]
    in_maps = []
    for core in range(NCORE):
        bb, g4 = core // 4, core % 4
        hq0, kv0 = g4 * 8, g4 * 2
        # weight retiling: [128, dc-major, head, col] contiguous per partition
        wq_t = wq_p[:, hq0:hq0 + HQ].reshape(NDC, 128, HQ * HD)
        wq_t = np.ascontiguousarray(wq_t.transpose(1, 0, 2)).reshape(128, NDC * HQ * HD)
        wk_t = wk_p[:, kv0:kv0 + HKV].reshape(NDC, 128, HKV * HD)
        wk_t = np.ascontiguousarray(wk_t.transpose(1, 0, 2)).reshape(128, NDC * HKV * HD)
        wv_t = wv[:, kv0 * HD:(kv0 + HKV) * HD].reshape(NDC, 128, HKV * HD)
        wv_t = np.ascontiguousarray(wv_t.transpose(1, 0, 2)).reshape(128, NDC * HKV * HD)
        # [p, n1, h, c]: per n1-tile, all 8 heads' [128,128] blocks contiguous
        wo_t = wo[hq0 * HD:(hq0 + HQ) * HD, :].reshape(HQ, 128, D // 128, 128)
        wo_t = np.ascontiguousarray(wo_t.transpose(1, 2, 0, 3)).reshape(128, D // 128 * HQ * 128)
        in_maps.append({
            "xT": xTs[bb],
            "wq": wq_t.astype(_BF16),
            "wk": wk_t.astype(_BF16),
            "wv": wv_t.astype(_BF16),
            "wo": wo_t.astype(_BF16),
            "cosT": cosT,
            "sinT": sinT,
            "cmask": cmask,
            "ones16": ones16,
            "ones32": ones32,
            "swp": swp,
        })
    return in_maps


def _run(in_maps, trace=False, tmpdir=None):
    from concourse.bass_utils import run_bass_kernel_spmd
    return run_bass_kernel_spmd(_graph(), in_maps, core_ids=list(range(NCORE)),
                                trace=trace, tmpdir=tmpdir)


def kernel(x, freqs_cos, freqs_sin, wq, wk, wv, wo):
    in_maps = _make_in_maps(np.asarray(x, np.float32),
                            np.asarray(freqs_cos, np.float32),
                            np.asarray(freqs_sin, np.float32),
                            np.asarray(wq, np.float32),
                            np.asarray(wk, np.float32),
                            np.asarray(wv, np.float32),
                            np.asarray(wo, np.float32))
    res = _run(in_maps).results
    outT = np.zeros((B, D, S), np.float32)
    for c in range(NCORE):
        outT[c // 4] += np.asarray(res[c]["out"], np.float32)
    return np.ascontiguousarray(outT.transpose(0, 2, 1))


def _ensure_ntff_hook():
    import types
    import ctypes
    import contextlib
    import antenv

    try:
        from antenv.axon_hooks import get_axon_ntff_profile_hook  # noqa: F401
        return
    except ImportError:
        pass
    so_path = "/opt/axon/libaxon_pjrt.so"
    if not os.path.exists(so_path):
        return
    lib = ctypes.CDLL(so_path)
    if not hasattr(lib, "axon_start_nrt_profile"):
        return
    lib.axon_start_nrt_profile.argtypes = [ctypes.POINTER(ctypes.c_int64),
                                           ctypes.c_size_t]
    lib.axon_start_nrt_profile.restype = ctypes.c_int64
    lib.axon_stop_nrt_profile.argtypes = [ctypes.c_char_p]
    lib.axon_stop_nrt_profile.restype = ctypes.c_int64

    @contextlib.contextmanager
    def _hook(output_dir, device_ids):
        import jax
        jax.devices()
        if device_ids:
            ids = (ctypes.c_int64 * len(device_ids))(*device_ids)
            rc = lib.axon_start_nrt_profile(ids, len(device_ids))
        else:
            rc = lib.axon_start_nrt_profile(None, 0)
        if rc != 0:
            raise RuntimeError(f"axon_start_nrt_profile rc={rc}")
        try:
            yield
        finally:
            n = lib.axon_stop_nrt_profile(str(output_dir).encode())
            print(f"profile: {n} ntff file(s) written to {output_dir}")

    mod = types.ModuleType("antenv.axon_hooks")
    hooks = {"h": _hook}
    mod.get_axon_ntff_profile_hook = lambda: hooks["h"]
    mod.set_axon_ntff_profile_hook = lambda h: hooks.__setitem__("h", h)
    sys.modules["antenv.axon_hooks"] = mod
    antenv.axon_hooks = mod


def profile_exec_ns(inputs, tmpdir=None):
    _ensure_ntff_hook()
    from concourse import bass_utils as _bu
    _bu.upload_artifacts = lambda d: f"local:{d}"
    in_maps = _make_in_maps(**{k: np.asarray(v, np.float32) for k, v in inputs.items()})
    r = _run(in_maps, trace=True, tmpdir=tmpdir)
    return r.exec_time_ns


# revision 25
# speedup vs baseline: 1.1981x; 1.0296x over previous
"""GQA causal attention (B=2,S=2048,D=4096,NH=32,NKV=8,HD=128) on 8 TRN2 cores.

Sharding: core c -> batch b=c//4, kv-group g=c%4 (2 kv heads, 8 q heads).
Each core computes a partial output x_b @ (its heads) @ wo_rows; host sums
the 4 partials per batch.

Device dataflow (bf16 matmuls, fp32 PSUM/softmax stats):
  phase 1: xT chunks [128,512] stream in; Q/K projections produce qT/kT in
           [HD,S] layout (+RoPE via a swap-permutation matmul and two DVE
           multiplies against [cos;cos], [-sin;sin] tables); V in [S,HD].
  phase 2: scores computed TRANSPOSED per 512-wide sq block:
           S_T[t,sq] = kT_tile.T @ qT_block, causal mask added from 4
           precomputed [128,512] tiles, exp -> P_T (bf16), rowsums via
           ones-vector matmul, attnT[d,sq] += (lhsT=V_tile) @ P_T,
           normalization = rank-1 (ones x 1/rowsum) matmul broadcast +
           one DVE multiply on the attnT PSUM->SBUF copy.
  phase 3: out[sq,:] = sum_h attnT_h.T @ wo_h, streamed to DRAM.
"""

import os
import sys

import numpy as np
import ml_dtypes

for _p in ("/opt/trn_rl_repo",):
    if os.path.isdir(_p) and _p not in sys.path:
        sys.path.insert(0, _p)

B, S, D = 2, 2048, 4096
NH, NKV, HD = 32, 8, 128
NCORE = 8
HQ = 8           # q heads per core
HKV = 2          # kv heads per core
NDC = D // 128   # 32 contraction chunks
SB = 512         # phase-1 seq block
NBLK = S // SB
NT = S // 128    # 16 sq tiles of 128
NB2 = S // 512   # phase-2 sq blocks of 512
SCALE = 1.0 / float(np.sqrt(HD))

_BF16 = ml_dtypes.bfloat16
_GRAPH = None


def _maybe_patch_ldw_opt():
    if os.environ.get("ATTN_LDW_OPT") != "1":
        return
    from concourse import bass_utils as _bu
    if getattr(_bu, "_ldw_patched", False):
        return
    _orig = _bu.run_command

    def _patched(argv, **kw):
        argv = ["--enable-ldw-opt=true" if a == "--enable-ldw-opt=false" else a
                for a in argv]
        return _orig(argv, **kw)

    _bu.run_command = _patched
    _bu._ldw_patched = True


def _build_graph():
    _maybe_patch_ldw_opt()
    import concourse.mybir as mybir
    import concourse.tile as tile
    from concourse import bacc

    f32 = mybir.dt.float32
    bf16 = mybir.dt.bfloat16
    AF = mybir.ActivationFunctionType
    OP = mybir.AluOpType

    nc = bacc.Bacc("TRN2", target_bir_lowering=False, debug=False)
    xT_d = nc.declare_dram_parameter("xT", [D, S], bf16, isOutput=False)
    wq_d = nc.declare_dram_parameter("wq", [128, NDC * HQ * HD], bf16, isOutput=False)
    wk_d = nc.declare_dram_parameter("wk", [128, NDC * HKV * HD], bf16, isOutput=False)
    wv_d = nc.declare_dram_parameter("wv", [128, NDC * HKV * HD], bf16, isOutput=False)
    wo_d = nc.declare_dram_parameter("wo", [128, HQ * D], bf16, isOutput=False)
    cos_d = nc.declare_dram_parameter("cosT", [128, S], bf16, isOutput=False)
    sin_d = nc.declare_dram_parameter("sinT", [128, S], bf16, isOutput=False)
    cm_d = nc.declare_dram_parameter("cmask", [128, 4 * 512], f32, isOutput=False)
    o16_d = nc.declare_dram_parameter("ones16", [128, 128], bf16, isOutput=False)
    o32_d = nc.declare_dram_parameter("ones32", [1, 128], f32, isOutput=False)
    swp_d = nc.declare_dram_parameter("swp", [128, 128], bf16, isOutput=False)
    out_d = nc.declare_dram_parameter("out", [D, S], f32, isOutput=True)

    with tile.TileContext(nc) as tc:
        with tc.tile_pool(name="res", bufs=1) as res:
            swp = res.tile([128, 128], bf16, tag="swp")
            nc.gpsimd.dma_start(out=swp[:], in_=swp_d[:])
            ones16 = res.tile([128, 128], bf16, tag="ones16")
            nc.gpsimd.dma_start(out=ones16[:], in_=o16_d[:])
            ones32 = res.tile([1, 128], f32, tag="ones32")
            nc.gpsimd.dma_start(out=ones32[:], in_=o32_d[:])
            res_qkv = tc.alloc_tile_pool(name="res_qkv", bufs=1)
            qT = [res_qkv.tile([128, S], bf16, tag=f"qT{h}", name=f"qT{h}") for h in range(HQ)]
            kT = [res_qkv.tile([128, S], bf16, tag=f"kT{g}", name=f"kT{g}") for g in range(HKV)]
            V = [res_qkv.tile([128, S], bf16, tag=f"V{g}", name=f"V{g}") for g in range(HKV)]

            # ---------------- Phase 1: QKV projections + RoPE ----------------
            with tc.tile_pool(name="p1w", bufs=1) as p1w, \
                 tc.tile_pool(name="p1x", bufs=33) as p1x, \
                 tc.tile_pool(name="p1s", bufs=2) as p1s, \
                 tc.tile_pool(name="ps1", bufs=3, space="PSUM") as ps1:
                for blk in range(NBLK):
                    s0 = blk * SB
                    xts = []
                    for dc in range(NDC):
                        t = p1x.tile([128, SB], bf16, tag="xT", name=f"xt{blk}_{dc}")
                        nc.sync.dma_start(
                            out=t[:], in_=xT_d[dc * 128:(dc + 1) * 128, s0:s0 + SB])
                        xts.append(t)
                    if blk == 0:
                        wq_sb = p1w.tile([128, NDC * HQ * HD], bf16, tag="wq")
                        qn = NDC * HQ * HD // 8
                        for q in range(8):
                            nc.gpsimd.dma_start(out=wq_sb[:, q * qn:(q + 1) * qn],
                                                in_=wq_d[:, q * qn:(q + 1) * qn])
                        wk_sb = p1w.tile([128, NDC * HKV * HD], bf16, tag="wk")
                        nc.scalar.dma_start(out=wk_sb[:], in_=wk_d[:])
                        wv_sb = p1w.tile([128, NDC * HKV * HD], bf16, tag="wv")
                        nc.scalar.dma_start(out=wv_sb[:], in_=wv_d[:])
                        cosb = p1w.tile([128, S], bf16, tag="cos")
                        nc.scalar.dma_start(out=cosb[:], in_=cos_d[:])
                        sinb = p1w.tile([128, S], bf16, tag="sin")
                        nc.scalar.dma_start(out=sinb[:], in_=sin_d[:])

                    # Q and K projections (transposed output [HD, seq]) + rope
                    for h in range(HQ + HKV):
                        ps = ps1.tile([128, SB], f32, tag="pp", bufs=3)
                        for dc in range(NDC):
                            if h < HQ:
                                w_sl = wq_sb[:, (dc * HQ + h) * 128:(dc * HQ + h + 1) * 128]
                            else:
                                gg = h - HQ
                                w_sl = wk_sb[:, (dc * HKV + gg) * 128:(dc * HKV + gg + 1) * 128]
                            nc.tensor.matmul(ps[:], w_sl, xts[dc][:],
                                             start=(dc == 0), stop=(dc == NDC - 1))
                        tmp = p1s.tile([128, SB], bf16, tag="ropetmp")
                        nc.vector.tensor_copy(tmp[:], ps[:])
                        qsw_ps = ps1.tile([128, SB], f32, tag="qsw", bufs=2)
                        nc.tensor.matmul(qsw_ps[:], swp[:], tmp[:], start=True, stop=True)
                        dst = qT[h] if h < HQ else kT[h - HQ]
                        dsl = dst[:, s0:s0 + SB]
                        ra = p1s.tile([128, SB], bf16, tag="ra")
                        rb = p1s.tile([128, SB], bf16, tag="rb")
                        nc.vector.tensor_tensor(ra[:], tmp[:], cosb[:, s0:s0 + SB], op=OP.mult)
                        nc.vector.tensor_tensor(rb[:], qsw_ps[:], sinb[:, s0:s0 + SB], op=OP.mult)
                        nc.vector.tensor_tensor(dsl[:], ra[:], rb[:], op=OP.add)
                    # V projection (natural [seq, d] layout)
                    for st in range(SB // 128):
                        ps = ps1.tile([128, HKV * HD], f32, tag="pv", bufs=2)
                        for dc in range(NDC):
                            nc.tensor.matmul(ps[:], xts[dc][:, st * 128:(st + 1) * 128],
                                             wv_sb[:, dc * 256:(dc + 1) * 256],
                                             start=(dc == 0), stop=(dc == NDC - 1))
                        sg = blk * (SB // 128) + st
                        for g in range(HKV):
                            nc.vector.tensor_copy(
                                V[g][:, sg * 128:(sg + 1) * 128],
                                ps[:, g * 128:(g + 1) * 128])

            # -------- Phase 2+3: attention (transposed scores) + out proj ---
            res_a = tc.alloc_tile_pool(name="res_a", bufs=1, side="right")
            attnT = [res_a.tile([128, S], bf16, tag=f"aT{h}", name=f"aT{h}")
                     for h in range(HQ)]
            with tc.tile_pool(name="p2c", bufs=1) as p2c, \
                 tc.tile_pool(name="p2", bufs=68) as p2, \
                 tc.tile_pool(name="p2s", bufs=4) as p2s, \
                 tc.tile_pool(name="p3w", bufs=4) as p3w, \
                 tc.tile_pool(name="p3o", bufs=4) as p3o, \
                 tc.tile_pool(name="pss", bufs=2, space="PSUM") as pss, \
                 tc.tile_pool(name="psr", bufs=1, space="PSUM") as psr, \
                 tc.tile_pool(name="psa", bufs=2, space="PSUM") as psa, \
                 tc.tile_pool(name="psb", bufs=1, space="PSUM") as psb, \
                 tc.tile_pool(name="pso", bufs=2, space="PSUM") as pso:
                cmask = p2c.tile([128, 4 * 512], f32, tag="cmask")
                nc.gpsimd.dma_start(out=cmask[:], in_=cm_d[:])
                for b in range(NB2):
                    ntt = 4 * (b + 1)
                    sq0 = b * 512
                    for g in range(HKV):
                        pts_h = {}
                        for hh in range(4):
                            h = g * 4 + hh
                            pts = []
                            for tt in range(ntt):
                                st_ps = pss.tile([128, 512], f32, tag="sc")
                                nc.tensor.matmul(st_ps[:],
                                                 kT[g][:, tt * 128:(tt + 1) * 128],
                                                 qT[h][:, sq0:sq0 + 512],
                                                 start=True, stop=True)
                                j = tt - 4 * b
                                if j >= 0:
                                    nc.vector.tensor_tensor(
                                        st_ps[:], st_ps[:],
                                        cmask[:, j * 512:(j + 1) * 512], op=OP.add)
                                pt = p2.tile([128, 512], bf16, tag="pt",
                                             name=f"pt{hh}_{tt}")
                                nc.scalar.activation(pt[:], st_ps[:], AF.Exp,
                                                     bias=0.0, scale=SCALE)
                                pts.append(pt)
                            pts_h[hh] = pts
                        for hh in range(4):
                            h = g * 4 + hh
                            pts = pts_h[hh]
                            rs_ps = psr.tile([1, 512], f32, tag="rs")
                            for tt in range(ntt):
                                nc.tensor.matmul(rs_ps[:], ones16[:, 0:1], pts[tt][:],
                                                 start=(tt == 0), stop=(tt == ntt - 1))
                            at_ps = psa.tile([128, 512], f32, tag="at")
                            for tt in range(ntt):
                                nc.tensor.matmul(at_ps[:],
                                                 V[g][:, tt * 128:(tt + 1) * 128],
                                                 pts[tt][:],
                                                 start=(tt == 0), stop=(tt == ntt - 1))
                            # normalize: broadcast rowsum, reciprocal, multiply
                            rs_sb = p2s.tile([1, 512], bf16, tag="rss")
                            nc.vector.tensor_copy(rs_sb[:], rs_ps[:])
                            bc_ps = psb.tile([128, 512], f32, tag="bc")
                            nc.tensor.matmul(bc_ps[:], ones16[0:1, :], rs_sb[:],
                                             start=True, stop=True)
                            rec = p2s.tile([128, 512], f32, tag="rec")
                            nc.vector.reciprocal(rec[:], bc_ps[:])
                            nc.vector.tensor_tensor(
                                attnT[h][:, sq0:sq0 + 512], at_ps[:], rec[:],
                                op=OP.mult)
                    # out projection for this sq block (all heads now final)
                    for n1 in range(D // 128):
                        wt = p3w.tile([128, HQ * HD], bf16, tag="wot",
                                      name=f"wot{n1}", bufs=8)
                        nc.gpsimd.dma_start(
                            out=wt[:], in_=wo_d[:, n1 * 1024:(n1 + 1) * 1024])
                        ps = pso.tile([128, 512], f32, tag="po")
                        for h in range(HQ):
                            nc.tensor.matmul(ps[:],
                                             wt[:, h * 128:(h + 1) * 128],
                                             attnT[h][:, sq0:sq0 + 512],
                                             start=(h == 0), stop=(h == HQ - 1))
                        ob = p3o.tile([128, 512], f32, tag="ob")
                        nc.vector.tensor_copy(ob[:], ps[:])
                        nc.sync.dma_start(
                            out=out_d[n1 * 128:(n1 + 1) * 128, sq0:sq0 + 512],
                            in_=ob[:])
            res_qkv.release()
            res_a.release()
    nc.compile()
    return nc


def _graph():
    global _GRAPH
    if _GRAPH is None:
        _GRAPH = _build_graph()
    return _GRAPH


_PERM = np.concatenate([np.arange(0, HD, 2), np.arange(1, HD, 2)])


def _make_in_maps(x, freqs_cos, freqs_sin, wq, wk, wv, wo):
    # deinterleave head-dim so rope pairs are (j, j+64) on partitions
    wq_p = wq.reshape(D, NH, HD)[:, :, _PERM]
    wk_p = wk.reshape(D, NKV, HD)[:, :, _PERM]
    cosT = np.concatenate([freqs_cos.T, freqs_cos.T], axis=0)
    cosT = np.ascontiguousarray(cosT).astype(_BF16)
    sinT = np.concatenate([-freqs_sin.T, freqs_sin.T], axis=0)
    sinT = np.ascontiguousarray(sinT).astype(_BF16)
    r = np.arange(128)[:, None]
    c = np.arange(512)[None, :]
    cmask = np.concatenate(
        [np.where(c < j * 128 + r, np.float32(-1e30), np.float32(0.0))
         for j in range(4)], axis=1).astype(np.float32)
    ones16 = np.ones((128, 128), dtype=_BF16)
    ones32 = np.ones((1, 128), dtype=np.float32)
    swp = np.zeros((128, 128), np.float32)
    swp[np.arange(128), (np.arange(128) + 64) % 128] = 1.0
    swp = swp.astype(_BF16)
    xTs = [np.ascontiguousarray(x[b].T).astype(_BF16) for b in range(B)]
    in_maps = []
    for core in range(NCORE):
        bb, g4 = core // 4, core % 4
        hq0, kv0 = g4 * 8, g4 * 2
        # weight retiling: [128, dc-major, head, col] contiguous per partition
        wq_t = wq_p[:, hq0:hq0 + HQ].reshape(NDC, 128, HQ * HD)
        wq_t = np.ascontiguousarray(wq_t.transpose(1, 0, 2)).reshape(128, NDC * HQ * HD)
        wk_t = wk_p[:, kv0:kv0 + HKV].reshape(NDC, 128, HKV * HD)
        wk_t = np.ascontiguousarray(wk_t.transpose(1, 0, 2)).reshape(128, NDC * HKV * HD)
        wv_t = wv[:, kv0 * HD:(kv0 + HKV) * HD].reshape(NDC, 128, HKV * HD)
        wv_t = np.ascontiguousarray(wv_t.transpose(1, 0, 2)).reshape(128, NDC * HKV * HD)
        # [p, n1, h, c]: per n1-tile, all 8 heads' [128,128] blocks contiguous
        wo_t = wo[hq0 * HD:(hq0 + HQ) * HD, :].reshape(HQ, 128, D // 128, 128)
        wo_t = np.ascontiguousarray(wo_t.transpose(1, 2, 0, 3)).reshape(128, D // 128 * HQ * 128)
        in_maps.append({
            "xT": xTs[bb],
            "wq": wq_t.astype(_BF16),
            "wk": wk_t.astype(_BF16),
            "wv": wv_t.astype(_BF16),
            "wo": wo_t.astype(_BF16),
            "cosT": cosT,
            "sinT": sinT,
            "cmask": cmask,
            "ones16": ones16,
            "ones32": ones32,
            "swp": swp,
        })
    return in_maps


def _run(in_maps, trace=False, tmpdir=None):
    from concourse.bass_utils import run_bass_kernel_spmd
    return run_bass_kernel_spmd(_graph(), in_maps, core_ids=list(range(NCORE)),
                                trace=trace, tmpdir=tmpdir)


def kernel(x, freqs_cos, freqs_sin, wq, wk, wv, wo):
    in_maps = _make_in_maps(np.asarray(x, np.float32),
                            np.asarray(freqs_cos, np.float32),
                            np.asarray(freqs_sin, np.float32),
                            np.asarray(wq, np.float32),
                            np.asarray(wk, np.float32),
                            np.asarray(wv, np.float32),
                            np.asarray(wo, np.float32))
    res = _run(in_maps).results
    outT = np.zeros((B, D, S), np.float32)
    for c in range(NCORE):
        outT[c // 4] += np.asarray(res[c]["out"], np.float32)
    return np.ascontiguousarray(outT.transpose(0, 2, 1))


def _ensure_ntff_hook():
    import types
    import ctypes
    import contextlib
    import antenv

    try:
        from antenv.axon_hooks import get_axon_ntff_profile_hook  # noqa: F401
        return
    except ImportError:
        pass
    so_path = "/opt/axon/libaxon_pjrt.so"
    if not os.path.exists(so_path):
        return
    lib = ctypes.CDLL(so_path)
    if not hasattr(lib, "axon_start_nrt_profile"):
        return
    lib.axon_start_nrt_profile.argtypes = [ctypes.POINTER(ctypes.c_int64),
                                           ctypes.c_size_t]
    lib.axon_start_nrt_profile.restype = ctypes.c_int64
    lib.axon_stop_nrt_profile.argtypes = [ctypes.c_char_p]
    lib.axon_stop_nrt_profile.restype = ctypes.c_int64

    @contextlib.contextmanager
    def _hook(output_dir, device_ids):
        import jax
        jax.devices()
        if device_ids:
            ids = (ctypes.c_int64 * len(device_ids))(*device_ids)
            rc = lib.axon_start_nrt_profile(ids, len(device_ids))
        else:
            rc = lib.axon_start_nrt_profile(None, 0)
        if rc != 0:
            raise RuntimeError(f"axon_start_nrt_profile rc={rc}")
        try:
            yield
        finally:
            n = lib.axon_stop_nrt_profile(str(output_dir).encode())
            print(f"profile: {n} ntff file(s) written to {output_dir}")

    mod = types.ModuleType("antenv.axon_hooks")
    hooks = {"h": _hook}
    mod.get_axon_ntff_profile_hook = lambda: hooks["h"]
    mod.set_axon_ntff_profile_hook = lambda h: hooks.__setitem__("h", h)
    sys.modules["antenv.axon_hooks"] = mod
    antenv.axon_hooks = mod


def profile_exec_ns(inputs, tmpdir=None):
    _ensure_ntff_hook()
    from concourse import bass_utils as _bu
    _bu.upload_artifacts = lambda d: f"local:{d}"
    in_maps = _make_in_maps(**{k: np.asarray(v, np.float32) for k, v in inputs.items()})
    r = _run(in_maps, trace=True, tmpdir=tmpdir)
    return r.exec_time_ns
